# revision 1
# baseline (speedup 1.0000x reference)
"""Trainium2 Bass kernel for a pre-norm multi-head attention block.

Problem: x(4,1024,768) -> LN -> QKV (12 heads x 64) -> softmax attention
-> out proj -> +residual.

Sharding: 8 cores = 4 batches x 2 head-groups (tensor parallel over heads).
Each core computes LN(x[b]) and 6 heads of attention, then a row-parallel
partial of the output projection.  The host sums the two partials per batch
(each core also adds 0.5*x + 0.5*proj_bias so the pair-sum reconstructs the
residual and bias exactly).

Matmuls run in bf16 (fp32 PSUM accumulation): on TRN2 this enables fast
weight load and keeps the PE array dense, ~3x faster than fp32/f32r paths.
"""

import sys

if "/opt/trn_rl_repo" not in sys.path:
    sys.path.insert(0, "/opt/trn_rl_repo")

import numpy as np

B = 4
N = 1024
DIM = 768
NHEAD = 12
DHEAD = 64
SCALE = DHEAD ** -0.5
G = 2                    # tensor-parallel groups
HPG = NHEAD // G         # heads per group = 6
DG = HPG * DHEAD         # feature dim per group = 384
DVH = DHEAD + 1          # v head width incl. ones column = 65
VW = HPG * DVH           # augmented v width = 390
NT = N // 128            # token tiles = 8
NC = DIM // 128          # input feature chunks = 6
NJ = DG // 128           # output feature chunks per group = 3

_PROGRAM = {}
LAST_RESULTS = None


def _install_profile_hook():
    """The agent image's ``antenv`` lacks ``axon_hooks``, which
    ``bass_utils`` needs for NTFF profiling under axon (BASS_TRACE=1).
    Recreate it from the slim ctypes implementation in trn_agent_boot."""
    import types
    if "antenv.axon_hooks" in sys.modules:
        return
    try:
        from trn_agent_boot.trn_boot import _ntff_profile_via_ctypes
        hook = _ntff_profile_via_ctypes("/opt/axon/libaxon_pjrt.so")
    except Exception:
        hook = None
    mod = types.ModuleType("antenv.axon_hooks")
    mod.get_axon_ntff_profile_hook = lambda: hook
    mod.set_axon_ntff_profile_hook = lambda h: None
    sys.modules["antenv.axon_hooks"] = mod
    try:
        import antenv
        antenv.axon_hooks = mod
    except Exception:
        pass


def _build_program(with_qk_bias=False):
    import concourse.bass as bass
    import concourse.tile as tile
    from concourse import mybir, bacc
    from concourse.masks import make_identity

    f32 = mybir.dt.float32
    bf16 = mybir.dt.bfloat16

    nc = bacc.Bacc(None)

    X = nc.dram_tensor("X", [N, DIM], f32, kind="ExternalInput")
    RES = nc.dram_tensor("RES", [N, DIM], f32, kind="ExternalInput")
    WQ = nc.dram_tensor("WQ", [128, NC, DG], bf16, kind="ExternalInput")
    WK = nc.dram_tensor("WK", [128, NC, DG], bf16, kind="ExternalInput")
    WVA = nc.dram_tensor("WVA", [128, NC, VW], bf16, kind="ExternalInput")
    WPT = nc.dram_tensor("WPT", [128, NJ, DIM], bf16, kind="ExternalInput")
    # [q_bias(384) | k_bias*SCALE(384) | v_bias_aug(390, 1.0 at ones cols) | ones(512)]
    QKVB = nc.dram_tensor("QKVB", [1, 2 * DG + VW + 512], bf16, kind="ExternalInput")
    OUT = nc.dram_tensor("OUT", [N, DIM], f32, kind="ExternalOutput")

    ONES_OFF = 2 * DG + VW

    Exp = mybir.ActivationFunctionType.Exp
    Log = mybir.ActivationFunctionType.Ln
    Sqrt = mybir.ActivationFunctionType.Sqrt
    Copy = mybir.ActivationFunctionType.Copy
    Square = mybir.ActivationFunctionType.Square
    sub = mybir.AluOpType.subtract
    mult = mybir.AluOpType.mult

    with tile.TileContext(nc) as tc:
        with (
            tc.tile_pool(name="consts", bufs=1) as consts,
            tc.tile_pool(name="xin", bufs=4) as xin_p,
            tc.tile_pool(name="stats", bufs=4) as stats_p,
            tc.tile_pool(name="xn", bufs=6) as xn_p,
            tc.tile_pool(name="big", bufs=1) as big_p,
            tc.tile_pool(name="expp", bufs=1) as exp_p,
            tc.tile_pool(name="sm", bufs=4) as sm_p,
            tc.tile_pool(name="resp", bufs=2) as res_p,
            tc.tile_pool(name="outp", bufs=2) as out_p,
            tc.tile_pool(name="partp", bufs=8) as part_p,
            tc.tile_pool(name="psmm", bufs=2, space="PSUM") as ps_mm,
            tc.tile_pool(name="pssc", bufs=3, space="PSUM") as ps_sc,
            tc.tile_pool(name="psav", bufs=3, space="PSUM") as ps_av,
        ):
            ident = consts.tile([128, 128], bf16, tag="ident")
            make_identity(nc, ident[:])

            wq_t = consts.tile([128, NC, DG], bf16, tag="wq")
            wk_t = consts.tile([128, NC, DG], bf16, tag="wk")
            wva_t = consts.tile([128, NC, VW], bf16, tag="wva")
            wpt_t = consts.tile([128, NJ, DIM], bf16, tag="wpt")
            qkvb_t = consts.tile([1, 2 * DG + VW + 512], bf16, tag="qkvb")
            nc.sync.dma_start(qkvb_t[:], QKVB[:])
            ones = qkvb_t[0:1, ONES_OFF:ONES_OFF + 512]

            xnT = big_p.tile([128, NT, NC, 128], bf16, tag="xnT")
            qT = big_p.tile([128, NJ, N], bf16, tag="qT")
            kT = big_p.tile([128, NJ, N], bf16, tag="kT")
            vaug = big_p.tile([128, NT, VW], bf16, tag="vaug")
            aoT = big_p.tile([128, NJ, N], bf16, tag="aoT")

            xn_tiles = [None] * NT

            def ln_tile(i):
                xt = xin_p.tile([128, DIM], f32, tag="xin")
                nc.sync.dma_start(xt[:], X[i * 128:(i + 1) * 128, :])
                mean = stats_p.tile([128, 1], f32, tag="mean")
                va = stats_p.tile([128, 1], f32, tag="va")
                if i % 2 == 0:
                    # stats via DVE bn_stats
                    st6 = stats_p.tile([128, 3, 6], f32, tag="st6")
                    for s in range(3):
                        nc.vector.bn_stats(st6[:, s, :],
                                           xt[:, s * 256:(s + 1) * 256])
                    mv = stats_p.tile([128, 2], f32, tag="mv")
                    nc.vector.bn_aggr(mv[:], st6[:])
                    nc.vector.tensor_copy(mean[:], mv[:, 0:1])
                    nc.vector.tensor_scalar_mul(va[:], mv[:, 1:2], float(DIM))
                else:
                    # stats via ScalarE accum_out (runs in parallel with the
                    # DVE-stats tiles during the intro)
                    scr = stats_p.tile([128, DIM], bf16, tag="scr")
                    s1 = stats_p.tile([128, 1], f32, tag="s1")
                    s2 = stats_p.tile([128, 1], f32, tag="s2")
                    nc.scalar.activation(scr[:], xt[:], Copy, accum_out=s1[:])
                    nc.scalar.activation(scr[:], xt[:], Square, accum_out=s2[:])
                    nc.vector.tensor_scalar_mul(mean[:], s1[:], 1.0 / DIM)
                    vv = stats_p.tile([128, 1], f32, tag="vv")
                    nc.vector.scalar_tensor_tensor(
                        out=vv[:], in0=mean[:], scalar=-1.0, in1=s1[:],
                        op0=mult, op1=mult)
                    nc.vector.tensor_add(va[:], vv[:], s2[:])
                # inv_std = exp(-0.5*ln(var)): ln and exp share one ScalarE
                # table set with the attention exps, so the table never
                # reloads mid-kernel (Sqrt lives in a different set)
                lnv = stats_p.tile([128, 1], f32, tag="lnv")
                nc.scalar.activation(lnv[:], va[:], Log,
                                     scale=1.0 / float(DIM - 1))
                inv = stats_p.tile([128, 1], f32, tag="inv")
                nc.scalar.activation(inv[:], lnv[:], Exp, scale=-0.5)
                xn = xn_p.tile([128, DIM], bf16, tag="xn")
                nc.vector.tensor_scalar(xn[:], xt[:], mean[:], inv[:],
                                        op0=sub, op1=mult)
                xn_tiles[i] = xn

            def transpose_tile(i):
                ptr = ps_mm.tile([128, NC * 128], bf16, tag="mm")
                for c in range(NC):
                    nc.tensor.transpose(
                        ptr[:, c * 128:(c + 1) * 128],
                        xn_tiles[i][:, c * 128:(c + 1) * 128],
                        ident[:])
                if i % 2 == 0:
                    nc.scalar.copy(xnT[:, i, :, :], ptr[:])
                else:
                    nc.vector.tensor_copy(xnT[:, i, :, :], ptr[:])

            def keep_warm(k):
                # dependency-free identity matmuls: fill PE idle windows so
                # the HAM clock gate stays at 2.4 GHz
                for _ in range(k):
                    nc.tensor.matmul(warm[:], ident[:], ident[:],
                                     start=True, stop=True)

            def qk_half(n, j, w_t, dst, boff):
                # single-half group (used for j=0 so it can start before
                # the second LN half is done)
                p = ps_mm.tile([128, 512], f32, tag="mm")
                for c in range(NC):
                    nc.tensor.matmul(p[:], w_t[:, c, j * 128:(j + 1) * 128],
                                     xnT[:, n * 4:(n + 1) * 4, c, :],
                                     start=(c == 0),
                                     stop=(c == NC - 1 and not with_qk_bias))
                if with_qk_bias:
                    nc.tensor.matmul(
                        p[:], qkvb_t[0:1, boff + j * 128:boff + (j + 1) * 128],
                        ones, start=False, stop=True)
                nc.scalar.copy(dst[:, j, n * 512:(n + 1) * 512], p[:])

            def qk_pair(j, w_t, dst, boff):
                # both n-halves off one weight load per chunk: the second
                # matmul reuses the stationary operand (cheap LDWEIGHTS)
                p0 = ps_mm.tile([128, 512], f32, tag="mm")
                p1 = ps_mm.tile([128, 512], f32, tag="mm")
                last = (not with_qk_bias)
                for c in range(NC):
                    lhs = w_t[:, c, j * 128:(j + 1) * 128]
                    nc.tensor.matmul(p0[:], lhs, xnT[:, 0:4, c, :],
                                     start=(c == 0),
                                     stop=(c == NC - 1 and last))
                    nc.tensor.matmul(p1[:], lhs, xnT[:, 4:8, c, :],
                                     start=(c == 0),
                                     stop=(c == NC - 1 and last))
                if with_qk_bias:
                    bias = qkvb_t[0:1, boff + j * 128:boff + (j + 1) * 128]
                    nc.tensor.matmul(p0[:], bias, ones, start=False, stop=True)
                    nc.tensor.matmul(p1[:], bias, ones, start=False, stop=True)
                nc.vector.tensor_copy(dst[:, j, 0:512], p0[:])
                nc.vector.tensor_copy(dst[:, j, 512:1024], p1[:])

            def v_tile(i):
                p = ps_mm.tile([128, VW], f32, tag="mm")
                for c in range(NC):
                    nc.tensor.matmul(p[:], xnT[:, i, c, :],
                                     wva_t[:, c, :], start=(c == 0),
                                     stop=(c == NC - 1 and not with_qk_bias))
                if with_qk_bias:
                    nc.tensor.matmul(p[:], ones[0:1, 0:128],
                                     qkvb_t[0:1, 2 * DG:2 * DG + VW],
                                     start=False, stop=True)
                nc.vector.tensor_copy(vaug[:, i, :], p[:])
                if not with_qk_bias:
                    # ones columns (softmax-sum trick) via cheap memset
                    nc.gpsimd.memset(vaug[:, i, DHEAD::DVH], 1.0)

            def head_scores(h, kc, dst, ns=(0, 1)):
                j = h // 2
                hp = (h % 2) * 64
                lhs = kT[hp:hp + 64, j, kc * 128:(kc + 1) * 128]
                for n in ns:
                    ps = ps_sc.tile([128, 512], f32, tag="sc")
                    nc.tensor.matmul(ps[:], lhs,
                                     qT[hp:hp + 64, j, n * 512:(n + 1) * 512],
                                     start=True, stop=True)
                    nc.scalar.activation(dst[:, kc, n * 512:(n + 1) * 512], ps[:], Exp)

            def head_av(h, expT, ns=(0, 1)):
                j = h // 2
                hp = (h % 2) * 64
                for n in ns:
                    pav = ps_av.tile([DVH, 512], f32, tag="av")
                    for kc in range(NT):
                        nc.tensor.matmul(pav[:], vaug[:, kc, h * DVH:(h + 1) * DVH],
                                         expT[:, kc, n * 512:(n + 1) * 512],
                                         start=(kc == 0), stop=(kc == NT - 1))
                    rs = sm_p.tile([1, 512], f32, tag="rsum")
                    nc.vector.tensor_copy(rs[:], pav[64:65, :])
                    rc = sm_p.tile([1, 512], f32, tag="recip")
                    nc.vector.reciprocal_approx_fast(rc[:], rs[:])
                    bc = sm_p.tile([64, 512], f32, tag="bcast")
                    nc.gpsimd.partition_broadcast(bc[:], rc[:])
                    nc.vector.tensor_mul(aoT[hp:hp + 64, j, n * 512:(n + 1) * 512],
                                         pav[0:64, :], bc[:])

            def pair_scores(t, eA, eB):
                hA, hB = 2 * t, 2 * t + 1
                for kc in range(NT):
                    head_scores(hA, kc, eA)
                    head_scores(hB, kc, eB)
                    keep_warm(8)

            # ---- output projection, two passes: head-pairs 0/1 (ready
            # early) accumulate into SBUF with the residual; the tail only
            # runs the last head-pair's matmuls ----
            parts = [None] * NT

            def proj_pass1(i):
                rt = res_p.tile([128, DIM], f32, tag="res")
                nc.sync.dma_start(rt[:], RES[i * 128:(i + 1) * 128, :])
                pt = part_p.tile([128, DIM], f32, tag="part")
                pp0 = ps_mm.tile([128, 512], f32, tag="mm")
                pp1 = ps_mm.tile([128, 256], f32, tag="mm")
                for c in range(2):
                    lhs = aoT[:, c, i * 128:(i + 1) * 128]
                    nc.tensor.matmul(pp0[:], lhs, wpt_t[:, c, 0:512],
                                     start=(c == 0), stop=(c == 1))
                    nc.tensor.matmul(pp1[:], lhs, wpt_t[:, c, 512:768],
                                     start=(c == 0), stop=(c == 1))
                nc.vector.tensor_add(pt[:, 0:512], pp0[:], rt[:, 0:512])
                nc.vector.tensor_add(pt[:, 512:768], pp1[:], rt[:, 512:768])
                parts[i] = pt

            def proj_pass2(i):
                ot = out_p.tile([128, DIM], f32, tag="out")
                pp0 = ps_mm.tile([128, 512], f32, tag="mm")
                pp1 = ps_mm.tile([128, 256], f32, tag="mm")
                lhs = aoT[:, 2, i * 128:(i + 1) * 128]
                nc.tensor.matmul(pp0[:], lhs, wpt_t[:, 2, 0:512],
                                 start=True, stop=True)
                nc.tensor.matmul(pp1[:], lhs, wpt_t[:, 2, 512:768],
                                 start=True, stop=True)
                nc.vector.tensor_add(ot[:, 0:512], pp0[:], parts[i][:, 0:512])
                nc.vector.tensor_add(ot[:, 512:768], pp1[:], parts[i][:, 512:768])
                nc.sync.dma_start(OUT[i * 128:(i + 1) * 128, :], ot[:])

            # ---- pipeline emission ----
            eA = exp_p.tile([128, NT, N], bf16, tag="expT", name="expTA")
            eB = exp_p.tile([128, NT, N], bf16, tag="expT2", name="expTB")

            # warm the PE clock (HAM) with dependency-free identity matmuls
            # so the real stream starts at 2.4 GHz
            warm = ps_sc.tile([128, 128], f32, tag="sc", name="warmps")

            for i in range(4):
                ln_tile(i)
                if i == 0:
                    nc.sync.dma_start(wq_t[:], WQ[:])
                    nc.sync.dma_start(wk_t[:], WK[:])
                transpose_tile(i)
                keep_warm(10)
            qk_half(0, 0, wq_t, qT, 0)
            qk_half(0, 0, wk_t, kT, DG)
            for i in range(4, NT):
                ln_tile(i)
                transpose_tile(i)
                # scores for the first K-half start streaming exps while the
                # second LN half is still running (kc 0-3 need only k-tokens
                # 0-511 and q-tokens 0-511, both ready after qk_half(0,0))
                kc = i - 4
                head_scores(0, kc, eA, ns=(0,))
                head_scores(1, kc, eB, ns=(0,))
                keep_warm(10)
            qk_half(1, 0, wq_t, qT, 0)
            qk_half(1, 0, wk_t, kT, DG)
            nc.sync.dma_start(wva_t[:], WVA[:])
            for kc in range(4):
                head_scores(0, kc, eA, ns=(1,))
                head_scores(1, kc, eB, ns=(1,))
                keep_warm(4)
            for kc in range(4, NT):
                head_scores(0, kc, eA)
                head_scores(1, kc, eB)
                keep_warm(4)
            for i in range(NT):
                v_tile(i)
            head_av(0, eA)
            head_av(1, eB)
            qk_pair(1, wq_t, qT, 0)
            qk_pair(1, wk_t, kT, DG)
            pair_scores(1, eA, eB)
            nc.sync.dma_start(wpt_t[:], WPT[:])
            head_av(2, eA)
            head_av(3, eB)
            qk_pair(2, wq_t, qT, 0)
            qk_pair(2, wk_t, kT, DG)
            for i in range(NT):
                proj_pass1(i)
            pair_scores(2, eA, eB)
            head_av(4, eA, ns=(0,))
            keep_warm(10)
            head_av(5, eB, ns=(0,))
            keep_warm(12)
            for i in range(4):
                proj_pass2(i)
            head_av(4, eA, ns=(1,))
            keep_warm(10)
            head_av(5, eB, ns=(1,))
            keep_warm(10)
            for i in range(4, NT):
                proj_pass2(i)

    nc.compile()
    return nc


def _get_program(with_qk_bias=False):
    if with_qk_bias not in _PROGRAM:
        _PROGRAM[with_qk_bias] = _build_program(with_qk_bias)
    return _PROGRAM[with_qk_bias]


def _prep_core_inputs(x_b, q_weight, k_weight, v_weight, q_bias, k_bias,
                      v_bias, g, bf16):
    f = np.float32
    sl = slice(g * DG, (g + 1) * DG)

    def chunked(wt, width, nchunks):
        # (768, width) -> (128, nchunks, width)
        return np.ascontiguousarray(
            wt.reshape(nchunks, 128, width).transpose(1, 0, 2)).astype(bf16)

    wq = chunked(np.ascontiguousarray(q_weight[sl, :].T, dtype=f), DG, NC)
    wk = chunked(np.ascontiguousarray((k_weight[sl, :] * SCALE).T, dtype=f), DG, NC)

    wv = np.ascontiguousarray(v_weight[sl, :].T, dtype=f)          # (768, 384)
    wva = np.zeros((DIM, VW), dtype=f)
    vba = np.zeros((VW,), dtype=f)
    for h in range(HPG):
        wva[:, h * DVH:h * DVH + DHEAD] = wv[:, h * DHEAD:(h + 1) * DHEAD]
        vba[h * DVH:h * DVH + DHEAD] = v_bias[sl][h * DHEAD:(h + 1) * DHEAD]
        vba[h * DVH + DHEAD] = 1.0
    wva = chunked(wva, VW, NC)

    qkvb = np.concatenate([
        q_bias[sl].astype(f), (k_bias[sl] * SCALE).astype(f), vba,
        np.ones((512,), dtype=f)])[None, :].astype(bf16)

    return {
        "X": np.ascontiguousarray(x_b, dtype=f),
        "WQ": wq, "WK": wk, "WVA": wva,
        "QKVB": np.ascontiguousarray(qkvb),
    }


def kernel(x, q_weight, k_weight, v_weight, q_bias, k_bias, v_bias,
           proj_weight, proj_bias, **_ignored):
    global LAST_RESULTS
    _install_profile_hook()
    import ml_dtypes
    from concourse.bass_utils import run_bass_kernel_spmd

    bf16 = ml_dtypes.bfloat16
    x = np.asarray(x, dtype=np.float32)
    q_weight = np.asarray(q_weight, dtype=np.float32)
    k_weight = np.asarray(k_weight, dtype=np.float32)
    v_weight = np.asarray(v_weight, dtype=np.float32)
    q_bias = np.asarray(q_bias, dtype=np.float32)
    k_bias = np.asarray(k_bias, dtype=np.float32)
    v_bias = np.asarray(v_bias, dtype=np.float32)
    proj_weight = np.asarray(proj_weight, dtype=np.float32)
    proj_bias = np.asarray(proj_bias, dtype=np.float32)

    with_qk_bias = bool(np.any(q_bias) or np.any(k_bias))
    nc = _get_program(with_qk_bias)

    wptT = proj_weight.T  # (din 768, dout 768)
    in_maps = []
    for b in range(B):
        res = (0.5 * x[b] + 0.5 * proj_bias[None, :]).astype(np.float32)
        for g in range(G):
            m = _prep_core_inputs(x[b], q_weight, k_weight, v_weight,
                                  q_bias, k_bias, v_bias, g, bf16)
            wpt_g = np.ascontiguousarray(wptT[g * DG:(g + 1) * DG, :],
                                         dtype=np.float32)  # (384, 768)
            m["WPT"] = np.ascontiguousarray(
                wpt_g.reshape(NJ, 128, DIM).transpose(1, 0, 2)).astype(bf16)
            m["RES"] = res
            in_maps.append(m)

    LAST_RESULTS = run_bass_kernel_spmd(nc, in_maps, core_ids=list(range(8)))
    outs = [LAST_RESULTS.results[c]["OUT"] for c in range(8)]
    full = np.stack([outs[2 * b] + outs[2 * b + 1] for b in range(B)], axis=0)
    return full.astype(np.float32)

